# revision 13
# baseline (speedup 1.0000x reference)
"""Trainium2 Bass kernel for nn_BlockLinear_MixerBlock (6-layer radix-4 butterfly mixer).

Math: 6 block-diagonal butterfly layers (radix 4, gaps 1..1024) over the
feature dim (4096) of x [8192, 4096] compose into a Monarch factorization:
layers 0-2 = dense 64x64 mix within each contiguous 64-chunk (A), layers
3-5 = dense 64x64 mix across stride-64 feature classes (B).

Device dataflow (data-parallel over 8 cores, 1024 batch cols/core), designed
so the PE does ONLY weight-stationary matmuls (no PE transposes at all):

  host:  xT = x.T as f16 [4096, 8192] (feature-major), sliced per core
  A   :  per feature tile t: psA[f',b] = WA_t.T @ xT_t   (lhsT=weights, N=512)
         evict f32->f16 into y1T (feature-major, 64KB/partition tile)
  shuf:  per class-pair m: z_m[64d+c, b] = y1T[64c+2m+d, b]
         -- pure SBUF->SBUF DMA row gather (2KB descriptors), no PE involved
  B   :  psB = WB_m.T @ z_m  (lhsT=weights, N=512), evict f16, DMA out to
         yT[64c''+2m+d, b] rows of the f16 feature-major output
  host:  y = concat(yT).T.astype(f32)

HBM traffic per core: 8.4 MB in + 8.4 MB out (f16); 8.4 MB SBUF<->SBUF
shuffle. PE: 128 matmuls N=512 per 1024 rows (~25 us) -- stays warm (no
transpose-mode ops). Evictions alternate DVE/ACT.

WA/WB are composed on the host from `weights` against the identity in
float64, stored f16 (measured end-to-end rel err ~5e-4, gate is 2e-2).
"""

import numpy as np

import concourse.bass as bass
import concourse.bacc as bacc
import concourse.mybir as mybir
from concourse.tile import TileContext
from concourse.bass_utils import run_bass_kernel_spmd

# ---- problem constants (hardcoded per contract) ----
N_CORES = 8
BS = 8192
D = 4096
BD = 4
NUM_LAYERS = 6
GAPS = [1, 4, 16, 64, 256, 1024]
BPC = BS // N_CORES          # 1024 batch cols per core
NFT = D // 128               # 32 feature tiles

F32 = mybir.dt.float32
F16 = mybir.dt.float16


# ---------------- host-side weight composition ----------------

def _ref_layers(x, weights, layers):
    bs = x.shape[0]
    y = x
    for i in layers:
        gap = GAPS[i]
        y = y.reshape(bs, -1, BD, gap).swapaxes(2, 3)
        y = y.reshape(bs, -1, BD)
        y = np.einsum('bnk,nkm->bnm', y, weights[i])
        y = y.reshape(bs, -1, gap, BD).swapaxes(2, 3)
    return y.reshape(bs, -1)


def _build_stage_weights(weights):
    w64 = weights.astype(np.float64)
    I = np.eye(D, dtype=np.float64)
    MA = _ref_layers(I, w64, [0, 1, 2])   # y1 = x @ MA (block-diag, 64-chunks)
    MB = _ref_layers(I, w64, [3, 4, 5])   # y  = y1 @ MB (block over stride-64)

    WA = np.zeros((128, D), np.float16)
    for t in range(NFT):
        WA[:, 128 * t:128 * (t + 1)] = MA[128 * t:128 * (t + 1), 128 * t:128 * (t + 1)]

    # lhsT for class pair (2m, 2m+1), with z partition order q = 64e + 32d + t
    # (c = 2t + e):  WB_m[q, 64d + c''] = MB[64c + 2m + d, 64c'' + 2m + d]
    MBr = MB.reshape(64, 64, 64, 64)      # [c, u', c'', u'']
    WB = np.zeros((128, D), np.float16)
    for m in range(NFT):
        for e in range(2):
            for dd in range(2):
                for t in range(32):
                    q = 64 * e + 32 * dd + t
                    WB[q, 128 * m + 64 * dd:128 * m + 64 * dd + 64] = \
                        MBr[2 * t + e, 2 * m + dd, :, 2 * m + dd]
    return WA, WB


# ---------------- device program ----------------

def _build_program(repeats=1, timing_io=False, ablate=()):
    ablate = frozenset(ablate)
    nc = bacc.Bacc("TRN2", target_bir_lowering=False, debug=False)
    if timing_io:
        # timing-only variant: big tensors live in device DRAM (no host I/O)
        xT_d = nc.dram_tensor("xT_int", [D, BPC], F16, kind="Internal")
        yT_d = nc.dram_tensor("yT_int", [D, BPC], F16, kind="Internal")
        yp_d = nc.dram_tensor("yprobe", [128, 4], F16, kind="ExternalOutput")
    else:
        xT_d = nc.dram_tensor("xT", [D, BPC], F16, kind="ExternalInput")
        yT_d = nc.dram_tensor("yT", [D, BPC], F16, kind="ExternalOutput")
        yp_d = None
    wa_d = nc.dram_tensor("wa", [128, D], F16, kind="ExternalInput")
    wb_d = nc.dram_tensor("wb", [128, D], F16, kind="ExternalInput")

    with TileContext(nc) as tc:
        with (
            tc.tile_pool(name="const", bufs=1) as const,
            tc.tile_pool(name="xin", bufs=4) as xin_pool,
            tc.tile_pool(name="y1", bufs=1) as y1_pool,
            tc.tile_pool(name="zbuf", bufs=2) as z_pool,
            tc.tile_pool(name="yout", bufs=4) as yo_pool,
            tc.tile_pool(name="psA", bufs=3, space="PSUM") as psA_pool,
            tc.tile_pool(name="psB", bufs=3, space="PSUM") as psB_pool,
        ):
            # prefetch feature-tile 0 of x before the 2 MiB of weight DMA so
            # the first matmuls are not stuck behind the weight loads
            x0 = None
            if repeats == 1 and not timing_io:
                x0 = xin_pool.tile([128, BPC], F16, name="x0", tag="xt")
                nc.sync.dma_start(x0[:], xT_d.ap()[0:128, :])
            wa_sb = const.tile([128, D], F16, name="wa_sb")
            wb_sb = const.tile([128, D], F16, name="wb_sb")
            for h in range(8):
                lo, hi = 512 * h, 512 * (h + 1)
                nc.sync.dma_start(wa_sb[:, lo:hi], wa_d.ap()[:, lo:hi])
                nc.sync.dma_start(wb_sb[:, lo:hi], wb_d.ap()[:, lo:hi])

            import contextlib
            if repeats > 1:
                assert repeats % 2 == 0
                # 2x unroll inside the HW loop so the two y1 buffers rotate:
                # iteration i+1's A-phase overlaps iteration i's B-phase
                with tc.For_i(0, repeats // 2, 1):
                    for u in range(2):
                        _body(nc, tc, xT_d, yT_d, wa_sb, wb_sb,
                              xin_pool, y1_pool, z_pool, yo_pool,
                              psA_pool, psB_pool, x0=None, ablate=ablate)
            else:
                _body(nc, tc, xT_d, yT_d, wa_sb, wb_sb,
                      xin_pool, y1_pool, z_pool, yo_pool,
                      psA_pool, psB_pool, x0=x0, ablate=ablate)
            if yp_d is not None:
                probe = const.tile([128, 4], F16, name="probe_sb")
                nc.sync.dma_start(probe[:], yT_d.ap()[0:128, 0:4])
                nc.sync.dma_start(yp_d.ap()[:, :], probe[:])
    nc.compile()
    return nc


def _body(nc, tc, xT_d, yT_d, wa_sb, wb_sb,
          xin_pool, y1_pool, z_pool, yo_pool, psA_pool, psB_pool, x0=None,
          ablate=frozenset()):

    def do_copy(k, out_ap, in_ap):
        # alternate PSUM-eviction copies between DVE and ACT
        if k % 2 == 0:
            nc.vector.tensor_copy(out_ap, in_ap)
        else:
            nc.scalar.copy(out_ap, in_ap)

    # y1T: feature-major stage-A output, f16 [128, 32*1024] (64 KiB/partition)
    y1 = y1_pool.tile([128, NFT * BPC], F16, name="y1T", tag="y1T")

    # ---- phase A: per feature tile t ----
    for t in range(NFT):
        if t == 0 and x0 is not None:
            xt = x0
        else:
            xt = xin_pool.tile([128, BPC], F16, name="xt", tag="xt")
            if "no_in" not in ablate:
                nc.sync.dma_start(xt[:], xT_d.ap()[128 * t:128 * (t + 1), :])
        if "no_amm" in ablate:
            continue
        for h in range(2):
            psA = psA_pool.tile([128, 512], F32, name="psA", tag="psA")
            nc.tensor.matmul(
                psA[:],
                lhsT=wa_sb[:, 128 * t:128 * (t + 1)],
                rhs=xt[:, 512 * h:512 * (h + 1)],
                start=True, stop=True,
            )
            do_copy(2 * t + h,
                    y1[:, BPC * t + 512 * h:BPC * t + 512 * (h + 1)], psA[:])

    # ---- phase B: in halves of 16 class-pairs m ----
    # y1T viewed [e, m, d, t, b]: partition 64e + 2m + d, free col 1024t+b
    y1v = y1[:].rearrange("(e m d) (t b) -> e m d t b", e=2, d=2, b=BPC)
    yTv = yT_d.ap().rearrange("(c u) b -> u c b", u=64)
    MH = NFT // 2
    for mh in range(2):
        # shuffle: Z[64e + 32d + t, 1024*(m-16mh) + b] = y1T[64e + 2m + d, 1024t + b]
        # one DMA per dst row q: reads 2KB from each of 16 partitions
        # (stride 2), writes one contiguous 32KB run; dst rows 0-63 live on
        # the even SDMA engines, rows 64-127 on the odd ones, so the per-row
        # DMAs drain engine-parallel.
        zh = z_pool.tile([128, MH * BPC], F16, name="zh", tag="zh")
        for e in range(2 if "no_shuf" not in ablate else 0):
            for dd in range(2):
                for t in range(NFT):
                    q = 64 * e + 32 * dd + t
                    src = y1v[e:e + 1, MH * mh:MH * (mh + 1),
                              dd:dd + 1, t:t + 1].squeeze()  # [m:16(P,stride2), b]
                    nc.sync.dma_start(zh[q:q + 1, :], src)
        for mm in range(MH):
            m = MH * mh + mm
            yo = yo_pool.tile([128, BPC], F16, name="yo", tag="yo")
            for h in range(2 if "no_bmm" not in ablate else 0):
                psB = psB_pool.tile([128, 512], F32, name="psB", tag="psB")
                nc.tensor.matmul(
                    psB[:],
                    lhsT=wb_sb[:, 128 * m:128 * (m + 1)],
                    rhs=zh[:, BPC * mm + 512 * h:BPC * mm + 512 * (h + 1)],
                    start=True, stop=True,
                )
                do_copy(2 * m + h + 1, yo[:, 512 * h:512 * (h + 1)], psB[:])
            for dd in range(2 if "no_out" not in ablate else 0):
                # yT[64c'' + 2m + d, b] = yo[64d + c'', b]
                dst = yTv[2 * m + dd:2 * m + dd + 1].squeeze()  # [c'':64, b]
                nc.gpsimd.dma_start(dst, yo[64 * dd:64 * dd + 64, :])


_PROGRAMS = {}


def _get_program(repeats=1):
    if repeats not in _PROGRAMS:
        _PROGRAMS[repeats] = _build_program(repeats)
    return _PROGRAMS[repeats]


def _run(x, weights, repeats=1, **spmd_kwargs):
    assert x.shape == (BS, D), x.shape
    WA, WB = _build_stage_weights(np.asarray(weights, dtype=np.float32))
    xT = np.ascontiguousarray(np.asarray(x, dtype=np.float16).T)   # [D, BS]
    nc = _get_program(repeats)
    in_maps = [
        {
            "xT": np.ascontiguousarray(xT[:, c * BPC:(c + 1) * BPC]),
            "wa": WA,
            "wb": WB,
        }
        for c in range(N_CORES)
    ]
    res = run_bass_kernel_spmd(nc, in_maps, core_ids=list(range(N_CORES)), **spmd_kwargs)
    yT = np.concatenate([res.results[c]["yT"] for c in range(N_CORES)], axis=1)
    return np.ascontiguousarray(yT.T).astype(np.float32), res


def kernel(x, weights):
    y, _ = _run(x, weights)
    return y


def _run_timing(weights, repeats, n_calls=6):
    """Delta-timing helper: runs the internal-I/O variant; returns wall times."""
    import time
    WA, WB = _build_stage_weights(np.asarray(weights, dtype=np.float32))
    key = ("timing", repeats)
    if key not in _PROGRAMS:
        _PROGRAMS[key] = _build_program(repeats, timing_io=True)
    nc = _PROGRAMS[key]
    in_maps = [{"wa": WA, "wb": WB} for _ in range(N_CORES)]
    walls = []
    for _ in range(n_calls):
        t0 = time.time()
        run_bass_kernel_spmd(nc, in_maps, core_ids=list(range(N_CORES)))
        walls.append(time.time() - t0)
    return walls
